# revision 7
# baseline (speedup 1.0000x reference)
"""Trainium2 Bass kernel for nn_CrossAttention (B=4, Sq=Skv=4096, E=1024, H=512).

Sharding: 8 cores = batch(4) x Sq-halves(2). Each core computes its full
[2048, 1024] output block independently (no collectives).

Per-core pipeline. The attention core (K/V projections, QK^T scores, softmax
numerator/denominator, PV) runs in fp8-e4m3 with DoubleRow perf mode (2x PE
throughput, contracting 256 per pass); errors injected before the softmax
average down over the 4096-key reduction. Wo/Wfc/Q-proj matmuls stay bf16
(their quantization error passes to the output at full strength).

  - kv loaded f32, PE-transposed (identity matmul), evacuated to fp8 kvT.
  - Wk/Wv pre-scaled by 16 and cast to fp8 (avoids fp8 subnormals); the 16x
    on K folds into the exp scale, the 16x on V folds into the 1/den
    reciprocal broadcast.
  - scores computed transposed ST[k, q] = KT^T QT in fp8 DoubleRow (pairs of
    h-tiles); softmax without per-row max: exp((st * SCALE/16) - C) with a
    global shift C chosen so max(ex) ~ 180 fits fp8-e4m3 (max 240).
  - exp -> fp8 ex pairs [P, 2, QB]; PV and the ones-vector denominator
    accumulate over 16 kt-pairs in PSUM via DoubleRow.
  - normalize with reciprocal broadcast (PE ones-broadcast, ones = 1/16),
    Wo + bias + residual (qT bf16), Wfc back to natural [q, f] layout,
    LayerNorm via bn_stats/bn_aggr, gamma/beta, DMA out f32.

The `repeat` build parameter traces the compute body R times (same data,
same outputs) inside one NEFF — used only for differential timing.
"""

import numpy as np


def _ensure_concourse():
    try:
        import concourse.bass  # noqa: F401
    except ImportError:
        import sys

        for p in ("/opt/trn_rl_repo", "/root/.axon_site/_ro/trn_rl_repo"):
            if p not in sys.path:
                sys.path.append(p)


_ensure_concourse()

from contextlib import ExitStack  # noqa: E402

import concourse.bacc as bacc  # noqa: E402
import concourse.mybir as mybir  # noqa: E402
import concourse.tile as tile  # noqa: E402
from concourse import bass_utils  # noqa: E402
from concourse.masks import make_identity  # noqa: E402

P = 128
E = 1024
EI = E // P  # 8
H = 512
HI = H // P  # 4
SQ = 2048  # q rows per core (Sq / 2)
SKV = 4096
QB = 512  # q block (moving free dim)
NQB = SQ // QB  # 4
NKT = SKV // P  # 32
KVC = 512  # kv chunk (token rows) for natural-load + transposed staging
SCALE = 1.0 / float(np.sqrt(512.0))
W8 = 16.0  # fp8 weight pre-scale (keeps Wk/Wv out of e4m3 subnormal range)
C_SHIFT = 3.7  # global exp shift; max scaled score ~8.9 -> max ex ~ e^5.2 = 180
EXP_SCALE = SCALE / W8

f32 = mybir.dt.float32
bf16 = mybir.dt.bfloat16
f8 = mybir.dt.float8e4
AF = mybir.ActivationFunctionType
ALU = mybir.AluOpType
DR = mybir.MatmulPerfMode.DoubleRow

_cached_nc = {}


def _build(repeat=1):
    if repeat in _cached_nc:
        return _cached_nc[repeat]

    nc = bacc.Bacc("TRN2")

    q_d = nc.dram_tensor("q_loc", [SQ, E], f32, kind="ExternalInput").ap()
    kv_d = nc.dram_tensor("kv_loc", [SKV, E], f32, kind="ExternalInput").ap()
    wq_d = nc.dram_tensor("Wq", [E, H], f32, kind="ExternalInput").ap()
    wk_d = nc.dram_tensor("Wk", [E, H], f32, kind="ExternalInput").ap()
    wv_d = nc.dram_tensor("Wv", [E, H], f32, kind="ExternalInput").ap()
    wo_d = nc.dram_tensor("Wo", [H, E], f32, kind="ExternalInput").ap()
    bo_d = nc.dram_tensor("bo", [E], f32, kind="ExternalInput").ap()
    wfc_d = nc.dram_tensor("Wfc", [E, E], f32, kind="ExternalInput").ap()
    g_d = nc.dram_tensor("ln_gamma", [E], f32, kind="ExternalInput").ap()
    b_d = nc.dram_tensor("ln_beta", [E], f32, kind="ExternalInput").ap()
    out_d = nc.dram_tensor("out_loc", [SQ, E], f32, kind="ExternalOutput").ap()

    with tile.TileContext(nc) as tc, ExitStack() as ctx:
        const = ctx.enter_context(tc.tile_pool(name="const", bufs=1))
        psum = ctx.enter_context(tc.tile_pool(name="psum", bufs=2, space="PSUM"))
        dram = ctx.enter_context(tc.tile_pool(name="dram", bufs=1, space="DRAM"))

        # ---------- persistent SBUF ----------
        wq_sb = const.tile([P, EI, H], bf16, name="wq_sb")
        wk8 = const.tile([P, EI, H], f8, name="wk8")  # 16*Wk, fp8
        wv8 = const.tile([P, EI, H], f8, name="wv8")  # 16*Wv, fp8
        wo_sb = const.tile([P, HI, E], bf16, name="wo_sb")
        wfc_sb = const.tile([P, EI, E], bf16, name="wfc_sb")
        bo_sb = const.tile([P, EI], f32, name="bo_sb")
        g128 = const.tile([P, E], f32, name="g128")
        b128 = const.tile([P, E], f32, name="b128")
        # [P, 2, 16] not [P, 2, 1]: dual-fp8 Ldweights needs the outer free
        # step even + 16B-aligned; only column 0 is used.
        ones8 = const.tile([P, 2, 16], f8, name="ones8")
        cbias = const.tile([P, 1], f32, name="cbias")
        eps_sb = const.tile([P, 1], f32, name="eps_sb")
        ones_f32 = const.tile([1, P], f32, name="ones_f32")
        id128 = const.tile([P, P], f32, name="id128")
        kt_sb = const.tile([P, HI, SKV], f8, name="kt_sb")  # 16*KT [h, k]
        v_sb = const.tile([P, NKT, H], f8, name="v_sb")  # 16*V  [k, h]

        nc.vector.memset(ones8, 1.0)
        nc.vector.memset(cbias, -C_SHIFT)
        nc.vector.memset(eps_sb, 1e-5)
        nc.vector.memset(ones_f32, 1.0 / W8)  # folds the 16x on V into 1/den
        make_identity(nc, id128)

        # bf16 copy of q in DRAM (SWDGE cast) for the xbar DMA-transpose; kv
        # is transposed on the PE instead (no cast pass needed)
        q_bf = dram.tile([SQ, E], bf16, name="q_bf")

        for _rep in range(repeat):
            # ---------- phase 1: K / V projections ----------
            with tc.tile_pool(name="p1", bufs=1) as p1:
                wk_bf = p1.tile([P, EI, H], bf16, name="wk_bf")
                wv_bf = p1.tile([P, EI, H], bf16, name="wv_bf")
                # SWDGE cast queue order = consumption order: wk/wv, q chunks
                # (+ wq), then tail-phase weights.
                nc.gpsimd.dma_start(wk_bf[:], wk_d.rearrange("(ei p) h -> p ei h", p=P))
                nc.gpsimd.dma_start(wv_bf[:], wv_d.rearrange("(ei p) h -> p ei h", p=P))
                nc.scalar.mul(wk8[:], wk_bf[:], W8)
                nc.vector.tensor_scalar_mul(wv8[:], wv_bf[:], W8)
                if _rep == 0:
                    nc.gpsimd.dma_start(q_bf[0:QB, :], q_d[0:QB, :])
                    nc.gpsimd.dma_start(
                        wq_sb[:], wq_d.rearrange("(ei p) h -> p ei h", p=P)
                    )
                    for c in range(1, NQB):
                        nc.gpsimd.dma_start(
                            q_bf[c * QB : (c + 1) * QB, :],
                            q_d[c * QB : (c + 1) * QB, :],
                        )
                    nc.gpsimd.dma_start(
                        wo_sb[:], wo_d.rearrange("(hj p) e -> p hj e", p=P)
                    )
                    nc.gpsimd.dma_start(
                        wfc_sb[:], wfc_d.rearrange("(ej p) f -> p ej f", p=P)
                    )
                    nc.sync.dma_start(bo_sb[:], bo_d.rearrange("(ej p) -> p ej", p=P))
                    nc.gpsimd.dma_start(
                        g128[:], g_d.rearrange("(a f) -> a f", a=1).broadcast_to((P, E))
                    )
                    nc.gpsimd.dma_start(
                        b128[:], b_d.rearrange("(a f) -> a f", a=1).broadcast_to((P, E))
                    )

                for c in range(SKV // KVC):
                    # natural f32 loads + PE transpose (4 tok-tiles into one
                    # PSUM bank) + one ACT/DVE evac per (chunk, e-slice)
                    kvn = [None] * 4
                    for t in range(4):
                        kvn[t] = p1.tile([P, E], f32, name="kvn", tag="kvn", bufs=8)
                        nc.sync.dma_start(
                            kvn[t][:], kv_d[c * KVC + t * P : c * KVC + (t + 1) * P, :]
                        )
                    kvt = p1.tile([P, EI, KVC], f8, name="kvt", tag="kvt", bufs=4)
                    for ei in range(EI):
                        tp = psum.tile([P, KVC], f32, name="tp", tag="ctx", bufs=4)
                        for t in range(4):
                            nc.tensor.matmul(
                                tp[:, t * P : (t + 1) * P],
                                kvn[t][:, ei * P : (ei + 1) * P],
                                id128[:],
                                is_transpose=True,
                                start=(t == 0),
                                stop=(t == 3),
                            )
                        if ei % 2 == 0:
                            nc.scalar.copy(kvt[:, ei : ei + 1, :], tp)
                        else:
                            nc.vector.tensor_copy(kvt[:, ei : ei + 1, :], tp)
                    # 16*KT[h, k] += (16Wk)[e,h]^T kvT[e,k]  (fp8 DoubleRow)
                    for hi in range(HI):
                        pk = psum.tile([P, KVC], f32, name="pk", tag="a", bufs=2)
                        for i in range(EI // 2):
                            nc.tensor.matmul(
                                pk,
                                wk8[:, 2 * i : 2 * i + 2, hi * P : (hi + 1) * P],
                                kvt[:, 2 * i : 2 * i + 2, :],
                                start=(i == 0),
                                stop=(i == EI // 2 - 1),
                                perf_mode=DR,
                            )
                        o = c * KVC
                        if hi % 2 == 0:
                            nc.scalar.copy(kt_sb[:, hi : hi + 1, o : o + KVC], pk)
                        else:
                            nc.vector.tensor_copy(
                                kt_sb[:, hi : hi + 1, o : o + KVC], pk
                            )
                    # 16*V[k, h] += kvT[e,k]^T (16Wv)[e,h]  (fp8 DoubleRow)
                    for kt in range(KVC // P):
                        pv = psum.tile([P, H], f32, name="pv", tag="a", bufs=2)
                        for i in range(EI // 2):
                            nc.tensor.matmul(
                                pv,
                                kvt[:, 2 * i : 2 * i + 2, kt * P : (kt + 1) * P],
                                wv8[:, 2 * i : 2 * i + 2, :],
                                start=(i == 0),
                                stop=(i == EI // 2 - 1),
                                perf_mode=DR,
                            )
                        g = c * (KVC // P) + kt
                        if kt % 2 == 0:
                            nc.scalar.copy(v_sb[:, g : g + 1, :], pv)
                        else:
                            nc.vector.tensor_copy(v_sb[:, g : g + 1, :], pv)

            # ---------- phase 2: attention + output per q block ----------
            with tc.tile_pool(name="p2", bufs=2) as p2:

                def q_proj(qb):
                    # qt transposes + QT[h, q] (bf16 matmul, fp8 evac)
                    qt = p2.tile([P, EI, QB], bf16, name="qt", tag="qt", bufs=2)
                    for ei in range(EI):
                        nc.sync.dma_start(
                            qt[:, ei : ei + 1, :],
                            q_bf[qb * QB : (qb + 1) * QB, ei * P : (ei + 1) * P],
                            transpose=True,
                        )
                    qt8 = p2.tile([P, HI, QB], f8, name="qt8", tag="qt8", bufs=2)
                    for hi in range(HI):
                        pq = psum.tile([P, QB], f32, name="pq", tag="a", bufs=2)
                        for ei in range(EI):
                            nc.tensor.matmul(
                                pq,
                                wq_sb[:, ei : ei + 1, hi * P : (hi + 1) * P],
                                qt[:, ei : ei + 1, :],
                                start=(ei == 0),
                                stop=(ei == EI - 1),
                            )
                        if hi % 2 == 0:
                            nc.scalar.copy(qt8[:, hi : hi + 1, :], pq)
                        else:
                            nc.vector.tensor_copy(qt8[:, hi : hi + 1, :], pq)
                    return qt, qt8

                pipe = {0: q_proj(0)}
                for qb in range(NQB):
                    qt, qt8 = pipe.pop(qb)

                    # attention: ST[k,q] -> exp -> fp8 pairs -> PV + denom
                    ctx_ps = [
                        psum.tile([P, QB], f32, name=f"cx{hj}", tag="ctx", bufs=4)
                        for hj in range(HI)
                    ]
                    den = psum.tile([1, QB], f32, name="den", tag="misc", bufs=2)
                    for pr in range(NKT // 2):
                        ex2 = p2.tile([P, 2, QB], f8, name="ex2", tag="ex", bufs=4)
                        for j in range(2):
                            kt = 2 * pr + j
                            st = psum.tile([P, QB], f32, name="st", tag="a", bufs=2)
                            for i in range(2):
                                nc.tensor.matmul(
                                    st,
                                    kt_sb[:, 2 * i : 2 * i + 2, kt * P : (kt + 1) * P],
                                    qt8[:, 2 * i : 2 * i + 2, :],
                                    start=(i == 0),
                                    stop=(i == 1),
                                    perf_mode=DR,
                                )
                            nc.scalar.activation(
                                ex2[:, j : j + 1, :],
                                st,
                                AF.Exp,
                                bias=cbias,
                                scale=EXP_SCALE,
                            )
                        for hj in range(HI):
                            nc.tensor.matmul(
                                ctx_ps[hj],
                                v_sb[:, 2 * pr : 2 * pr + 2, hj * P : (hj + 1) * P],
                                ex2[:],
                                start=(pr == 0),
                                stop=(pr == NKT // 2 - 1),
                                perf_mode=DR,
                            )
                        nc.tensor.matmul(
                            den,
                            ones8[:, :, 0:1],
                            ex2[:],
                            start=(pr == 0),
                            stop=(pr == NKT // 2 - 1),
                            perf_mode=DR,
                        )

                    # prefetch next block's Q projection to keep PE busy
                    # through the normalize chain below
                    if qb + 1 < NQB:
                        pipe[qb + 1] = q_proj(qb + 1)

                    # evacuate raw (unnormalized) ctx' as bf16 so Wo can start
                    # immediately; the 1/den scaling is applied after Wo (Wo is
                    # linear in ctx), off the PE critical path.
                    ctxb = []
                    for hj in range(HI):
                        cb = p2.tile([P, QB], bf16, name="cb", tag="ctxb", bufs=8)
                        if hj % 2 == 0:
                            nc.scalar.copy(cb, ctx_ps[hj])
                        else:
                            nc.vector.tensor_copy(cb, ctx_ps[hj])
                        ctxb.append(cb)
                    rec1 = p2.tile([1, QB], f32, name="rec1", tag="rec1", bufs=2)
                    nc.vector.reciprocal(rec1, den)
                    rps = psum.tile([P, QB], f32, name="rps", tag="a", bufs=2)
                    nc.tensor.matmul(rps, ones_f32[:], rec1)
                    rec128 = p2.tile([P, QB], f32, name="rec128", tag="rec128", bufs=2)
                    nc.scalar.copy(rec128, rps)

                    # Wo then scale by 1/(16 den), + bias + residual -> y^T [e, q]
                    ys = []
                    for ej in range(EI):
                        po = psum.tile([P, QB], f32, name="po", tag="a", bufs=2)
                        for hj in range(HI):
                            nc.tensor.matmul(
                                po,
                                wo_sb[:, hj : hj + 1, ej * P : (ej + 1) * P],
                                ctxb[hj],
                                start=(hj == 0),
                                stop=(hj == HI - 1),
                            )
                        yn = p2.tile([P, QB], f32, name="yn", tag="yn", bufs=3)
                        nc.vector.tensor_tensor(yn, po, rec128, op=ALU.mult)
                        y1 = p2.tile([P, QB], bf16, name="y1", tag="y1", bufs=3)
                        nc.scalar.add(y1, yn, bo_sb[:, ej : ej + 1])
                        y = p2.tile([P, QB], bf16, name="y", tag="y", bufs=16)
                        nc.vector.tensor_tensor(y, y1, qt[:, ej : ej + 1, :], op=ALU.add)
                        ys.append(y)

                    # Wfc back to natural [q, f], then LayerNorm + out
                    for qi in range(QB // P):
                        o2 = p2.tile([P, E], f32, name="o2", tag="o2", bufs=2)
                        for fj in range(2):
                            pf = psum.tile([P, H], f32, name="pf", tag="a", bufs=2)
                            for ej in range(EI):
                                nc.tensor.matmul(
                                    pf,
                                    ys[ej][:, qi * P : (qi + 1) * P],
                                    wfc_sb[:, ej : ej + 1, fj * H : (fj + 1) * H],
                                    start=(ej == 0),
                                    stop=(ej == EI - 1),
                                )
                            nc.vector.tensor_copy(o2[:, fj * H : (fj + 1) * H], pf)
                        st6 = p2.tile([P, 2, 6], f32, name="st6", tag="st6", bufs=3)
                        for g in range(2):
                            nc.vector.bn_stats(
                                st6[:, g : g + 1, :], o2[:, g * H : (g + 1) * H]
                            )
                        st2 = p2.tile([P, 2], f32, name="st2", tag="st2", bufs=3)
                        nc.vector.bn_aggr(st2, st6.rearrange("p a b -> p (a b)"))
                        stdt = p2.tile([P, 1], f32, name="stdt", tag="stdt", bufs=3)
                        nc.scalar.activation(stdt, st2[:, 1:2], AF.Sqrt, bias=eps_sb)
                        rstd = p2.tile([P, 1], f32, name="rstd", tag="rstd", bufs=3)
                        nc.vector.reciprocal(rstd, stdt)
                        nmr = p2.tile([P, 1], f32, name="nmr", tag="nmr", bufs=3)
                        nc.vector.tensor_tensor(nmr, st2[:, 0:1], rstd, op=ALU.mult)
                        nc.vector.tensor_scalar_mul(nmr, nmr, -1.0)
                        nrm = p2.tile([P, E], f32, name="nrm", tag="nrm", bufs=2)
                        nc.scalar.activation(nrm, o2, AF.Identity, bias=nmr, scale=rstd)
                        outt = p2.tile([P, E], f32, name="outt", tag="outt", bufs=2)
                        nc.vector.tensor_tensor(outt, nrm, g128, op=ALU.mult)
                        nc.vector.tensor_tensor(outt, outt, b128, op=ALU.add)
                        r0 = qb * QB + qi * P
                        nc.sync.dma_start(out_d[r0 : r0 + P, :], outt)

    nc.compile()
    _cached_nc[repeat] = nc
    return nc


def _in_maps(q_feat, kv_feat, Wq, Wk, Wv, Wo, bo, Wfc, ln_gamma, ln_beta):
    maps = []
    for c in range(8):
        b, half = c // 2, c % 2
        maps.append(
            {
                "q_loc": np.ascontiguousarray(
                    q_feat[b, half * SQ : (half + 1) * SQ], dtype=np.float32
                ),
                "kv_loc": np.ascontiguousarray(kv_feat[b], dtype=np.float32),
                "Wq": np.asarray(Wq, np.float32),
                "Wk": np.asarray(Wk, np.float32),
                "Wv": np.asarray(Wv, np.float32),
                "Wo": np.asarray(Wo, np.float32),
                "bo": np.asarray(bo, np.float32),
                "Wfc": np.asarray(Wfc, np.float32),
                "ln_gamma": np.asarray(ln_gamma, np.float32),
                "ln_beta": np.asarray(ln_beta, np.float32),
            }
        )
    return maps


def run_spmd(inputs, repeat=1, **kwargs):
    """Run the SPMD kernel; returns (full_output, BassKernelResults)."""
    nc = _build(repeat)
    maps = _in_maps(**inputs)
    res = bass_utils.run_bass_kernel_spmd(nc, maps, core_ids=list(range(8)), **kwargs)
    out = np.empty((4, 2 * SQ, E), np.float32)
    for c in range(8):
        b, half = c // 2, c % 2
        out[b, half * SQ : (half + 1) * SQ] = res.results[c]["out_loc"]
    return out, res


def kernel(**inputs):
    out, _ = run_spmd(inputs)
    return out


# revision 8
# speedup vs baseline: 1.1199x; 1.1199x over previous
"""Trainium2 Bass kernel for nn_CrossAttention (B=4, Sq=Skv=4096, E=1024, H=512).

Sharding: 8 cores = batch(4) x Sq-halves(2). Each core computes its full
[2048, 1024] output block independently (no collectives).

Per-core pipeline. The attention core (K/V projections, QK^T scores, softmax
numerator/denominator, PV) runs in fp8-e4m3 with DoubleRow perf mode (2x PE
throughput, contracting 256 per pass); errors injected before the softmax
average down over the 4096-key reduction. Wo/Wfc/Q-proj matmuls stay bf16
(their quantization error passes to the output at full strength).

  - kv loaded f32, PE-transposed (identity matmul), evacuated to fp8 kvT.
  - Wk/Wv pre-scaled by 16 and cast to fp8 (avoids fp8 subnormals); the 16x
    on K folds into the exp scale, the 16x on V folds into the 1/den
    reciprocal broadcast.
  - scores computed transposed ST[k, q] = KT^T QT in fp8 DoubleRow (pairs of
    h-tiles); softmax without per-row max: exp((st * SCALE/16) - C) with a
    global shift C chosen so max(ex) ~ 180 fits fp8-e4m3 (max 240).
  - exp -> fp8 ex pairs [P, 2, QB]; PV and the ones-vector denominator
    accumulate over 16 kt-pairs in PSUM via DoubleRow.
  - normalize with reciprocal broadcast (PE ones-broadcast, ones = 1/16),
    Wo + bias + residual (qT bf16), Wfc back to natural [q, f] layout,
    LayerNorm via bn_stats/bn_aggr, gamma/beta, DMA out f32.

The `repeat` build parameter traces the compute body R times (same data,
same outputs) inside one NEFF — used only for differential timing.
"""

import numpy as np


def _ensure_concourse():
    try:
        import concourse.bass  # noqa: F401
    except ImportError:
        import sys

        for p in ("/opt/trn_rl_repo", "/root/.axon_site/_ro/trn_rl_repo"):
            if p not in sys.path:
                sys.path.append(p)


_ensure_concourse()

from contextlib import ExitStack  # noqa: E402

import concourse.bacc as bacc  # noqa: E402
import concourse.mybir as mybir  # noqa: E402
import concourse.tile as tile  # noqa: E402
from concourse import bass_utils  # noqa: E402
from concourse.masks import make_identity  # noqa: E402

P = 128
E = 1024
EI = E // P  # 8
H = 512
HI = H // P  # 4
SQ = 2048  # q rows per core (Sq / 2)
SKV = 4096
QB = 512  # q block (moving free dim)
NQB = SQ // QB  # 4
NKT = SKV // P  # 32
KVC = 512  # kv chunk (token rows) for natural-load + transposed staging
SCALE = 1.0 / float(np.sqrt(512.0))
W8 = 16.0  # fp8 weight pre-scale (keeps Wk/Wv out of e4m3 subnormal range)
C_SHIFT = 4.3  # global exp shift; max scaled score ~8.9 -> max ex ~ e^4.6 = 100
EXP_SCALE = SCALE / W8

f32 = mybir.dt.float32
bf16 = mybir.dt.bfloat16
f8 = mybir.dt.float8e4
AF = mybir.ActivationFunctionType
ALU = mybir.AluOpType
DR = mybir.MatmulPerfMode.DoubleRow

_cached_nc = {}


def _build(repeat=1):
    if repeat in _cached_nc:
        return _cached_nc[repeat]

    nc = bacc.Bacc("TRN2")

    q_d = nc.dram_tensor("q_loc", [SQ, E], f32, kind="ExternalInput").ap()
    kv_d = nc.dram_tensor("kv_loc", [SKV, E], f32, kind="ExternalInput").ap()
    wq_d = nc.dram_tensor("Wq", [E, H], f32, kind="ExternalInput").ap()
    wk_d = nc.dram_tensor("Wk", [E, H], f32, kind="ExternalInput").ap()
    wv_d = nc.dram_tensor("Wv", [E, H], f32, kind="ExternalInput").ap()
    wo_d = nc.dram_tensor("Wo", [H, E], f32, kind="ExternalInput").ap()
    bo_d = nc.dram_tensor("bo", [E], f32, kind="ExternalInput").ap()
    wfc_d = nc.dram_tensor("Wfc", [E, E], f32, kind="ExternalInput").ap()
    g_d = nc.dram_tensor("ln_gamma", [E], f32, kind="ExternalInput").ap()
    b_d = nc.dram_tensor("ln_beta", [E], f32, kind="ExternalInput").ap()
    out_d = nc.dram_tensor("out_loc", [SQ, E], f32, kind="ExternalOutput").ap()

    with tile.TileContext(nc) as tc, ExitStack() as ctx:
        const = ctx.enter_context(tc.tile_pool(name="const", bufs=1))
        psum = ctx.enter_context(tc.tile_pool(name="psum", bufs=2, space="PSUM"))
        dram = ctx.enter_context(tc.tile_pool(name="dram", bufs=1, space="DRAM"))

        # ---------- persistent SBUF ----------
        wq_sb = const.tile([P, EI, H], bf16, name="wq_sb")
        wk8 = const.tile([P, EI, H], f8, name="wk8")  # 16*Wk, fp8
        wv8 = const.tile([P, EI, H], f8, name="wv8")  # 16*Wv, fp8
        wo_sb = const.tile([P, HI, E], bf16, name="wo_sb")
        wfc_sb = const.tile([P, EI, E], bf16, name="wfc_sb")
        bo_sb = const.tile([P, EI], f32, name="bo_sb")
        g128 = const.tile([P, E], f32, name="g128")
        b128 = const.tile([P, E], f32, name="b128")
        # [P, 2, 16] not [P, 2, 1]: dual-fp8 Ldweights needs the outer free
        # step even + 16B-aligned; only column 0 is used.
        ones8 = const.tile([P, 2, 16], f8, name="ones8")
        cbias = const.tile([P, 1], f32, name="cbias")
        eps_sb = const.tile([P, 1], f32, name="eps_sb")
        ones_f32 = const.tile([1, P], f32, name="ones_f32")
        id128 = const.tile([P, P], f32, name="id128")
        kt_sb = const.tile([P, HI, SKV], f8, name="kt_sb")  # 16*KT [h, k]
        v_sb = const.tile([P, NKT, H], f8, name="v_sb")  # 16*V  [k, h]

        nc.vector.memset(ones8, 1.0)
        nc.vector.memset(cbias, -C_SHIFT)
        nc.vector.memset(eps_sb, 1e-5)
        nc.vector.memset(ones_f32, 1.0 / W8)  # folds the 16x on V into 1/den
        make_identity(nc, id128)

        # bf16 copy of q in DRAM (SWDGE cast) for the xbar DMA-transpose; kv
        # is transposed on the PE instead (no cast pass needed)
        q_bf = dram.tile([SQ, E], bf16, name="q_bf")

        for _rep in range(repeat):
            # ---------- phase 1: K / V projections ----------
            with tc.tile_pool(name="p1", bufs=1) as p1:
                wk_bf = p1.tile([P, EI, H], bf16, name="wk_bf")
                wv_bf = p1.tile([P, EI, H], bf16, name="wv_bf")
                # SWDGE cast queue order = consumption order: wk/wv, q chunks
                # (+ wq), then tail-phase weights.
                nc.gpsimd.dma_start(wk_bf[:], wk_d.rearrange("(ei p) h -> p ei h", p=P))
                nc.gpsimd.dma_start(wv_bf[:], wv_d.rearrange("(ei p) h -> p ei h", p=P))
                nc.scalar.mul(wk8[:], wk_bf[:], W8)
                nc.vector.tensor_scalar_mul(wv8[:], wv_bf[:], W8)
                if _rep == 0:
                    nc.gpsimd.dma_start(q_bf[0:QB, :], q_d[0:QB, :])
                    nc.gpsimd.dma_start(
                        wq_sb[:], wq_d.rearrange("(ei p) h -> p ei h", p=P)
                    )
                    for c in range(1, NQB):
                        nc.gpsimd.dma_start(
                            q_bf[c * QB : (c + 1) * QB, :],
                            q_d[c * QB : (c + 1) * QB, :],
                        )
                    nc.gpsimd.dma_start(
                        wo_sb[:], wo_d.rearrange("(hj p) e -> p hj e", p=P)
                    )
                    nc.gpsimd.dma_start(
                        wfc_sb[:], wfc_d.rearrange("(ej p) f -> p ej f", p=P)
                    )
                    nc.sync.dma_start(bo_sb[:], bo_d.rearrange("(ej p) -> p ej", p=P))
                    nc.gpsimd.dma_start(
                        g128[:], g_d.rearrange("(a f) -> a f", a=1).broadcast_to((P, E))
                    )
                    nc.gpsimd.dma_start(
                        b128[:], b_d.rearrange("(a f) -> a f", a=1).broadcast_to((P, E))
                    )

                for c in range(SKV // KVC):
                    # natural f32 loads + PE transpose (4 tok-tiles into one
                    # PSUM bank) + one ACT/DVE evac per (chunk, e-slice)
                    kvn = [None] * 4
                    for t in range(4):
                        kvn[t] = p1.tile([P, E], f32, name="kvn", tag="kvn", bufs=8)
                        nc.sync.dma_start(
                            kvn[t][:], kv_d[c * KVC + t * P : c * KVC + (t + 1) * P, :]
                        )
                    kvt = p1.tile([P, EI, KVC], f8, name="kvt", tag="kvt", bufs=4)
                    for ei in range(EI):
                        tp = psum.tile([P, KVC], f32, name="tp", tag="ctx", bufs=4)
                        for t in range(4):
                            nc.tensor.matmul(
                                tp[:, t * P : (t + 1) * P],
                                kvn[t][:, ei * P : (ei + 1) * P],
                                id128[:],
                                is_transpose=True,
                                start=(t == 0),
                                stop=(t == 3),
                            )
                        if ei % 2 == 0:
                            nc.scalar.copy(kvt[:, ei : ei + 1, :], tp)
                        else:
                            nc.vector.tensor_copy(kvt[:, ei : ei + 1, :], tp)
                    # 16*KT[h, k] += (16Wk)[e,h]^T kvT[e,k]  (fp8 DoubleRow)
                    for hi in range(HI):
                        pk = psum.tile([P, KVC], f32, name="pk", tag="a", bufs=3)
                        for i in range(EI // 2):
                            nc.tensor.matmul(
                                pk,
                                wk8[:, 2 * i : 2 * i + 2, hi * P : (hi + 1) * P],
                                kvt[:, 2 * i : 2 * i + 2, :],
                                start=(i == 0),
                                stop=(i == EI // 2 - 1),
                                perf_mode=DR,
                            )
                        o = c * KVC
                        if hi % 2 == 0:
                            nc.scalar.copy(kt_sb[:, hi : hi + 1, o : o + KVC], pk)
                        else:
                            nc.vector.tensor_copy(
                                kt_sb[:, hi : hi + 1, o : o + KVC], pk
                            )
                    # 16*V[k, h] += kvT[e,k]^T (16Wv)[e,h]  (fp8 DoubleRow)
                    for kt in range(KVC // P):
                        pv = psum.tile([P, H], f32, name="pv", tag="a", bufs=3)
                        for i in range(EI // 2):
                            nc.tensor.matmul(
                                pv,
                                kvt[:, 2 * i : 2 * i + 2, kt * P : (kt + 1) * P],
                                wv8[:, 2 * i : 2 * i + 2, :],
                                start=(i == 0),
                                stop=(i == EI // 2 - 1),
                                perf_mode=DR,
                            )
                        g = c * (KVC // P) + kt
                        if kt % 2 == 0:
                            nc.scalar.copy(v_sb[:, g : g + 1, :], pv)
                        else:
                            nc.vector.tensor_copy(v_sb[:, g : g + 1, :], pv)

            # ---------- phase 2: attention + output per q block ----------
            with tc.tile_pool(name="p2", bufs=2) as p2:

                def q_proj(qb):
                    # qt transposes + QT[h, q] (bf16 matmul, fp8 evac)
                    qt = p2.tile([P, EI, QB], bf16, name="qt", tag="qt", bufs=2)
                    for ei in range(EI):
                        nc.sync.dma_start(
                            qt[:, ei : ei + 1, :],
                            q_bf[qb * QB : (qb + 1) * QB, ei * P : (ei + 1) * P],
                            transpose=True,
                        )
                    qt8 = p2.tile([P, HI, QB], f8, name="qt8", tag="qt8", bufs=2)
                    for hi in range(HI):
                        pq = psum.tile([P, QB], f32, name="pq", tag="a", bufs=3)
                        for ei in range(EI):
                            nc.tensor.matmul(
                                pq,
                                wq_sb[:, ei : ei + 1, hi * P : (hi + 1) * P],
                                qt[:, ei : ei + 1, :],
                                start=(ei == 0),
                                stop=(ei == EI - 1),
                            )
                        if hi % 2 == 0:
                            nc.scalar.copy(qt8[:, hi : hi + 1, :], pq)
                        else:
                            nc.vector.tensor_copy(qt8[:, hi : hi + 1, :], pq)
                    return qt, qt8

                pipe = {0: q_proj(0)}
                for qb in range(NQB):
                    qt, qt8 = pipe.pop(qb)

                    # attention: ST[k,q] -> exp -> fp8 pairs -> PV + denom
                    ctx_ps = [
                        psum.tile([P, QB], f32, name=f"cx{hj}", tag="ctx", bufs=4)
                        for hj in range(HI)
                    ]
                    den = psum.tile([1, QB], f32, name="den", tag="misc", bufs=1)
                    for pr in range(NKT // 2):
                        ex2 = p2.tile([P, 2, QB], f8, name="ex2", tag="ex", bufs=6)
                        for j in range(2):
                            kt = 2 * pr + j
                            st = psum.tile([P, QB], f32, name="st", tag="a", bufs=3)
                            for i in range(2):
                                nc.tensor.matmul(
                                    st,
                                    kt_sb[:, 2 * i : 2 * i + 2, kt * P : (kt + 1) * P],
                                    qt8[:, 2 * i : 2 * i + 2, :],
                                    start=(i == 0),
                                    stop=(i == 1),
                                    perf_mode=DR,
                                )
                            nc.scalar.activation(
                                ex2[:, j : j + 1, :],
                                st,
                                AF.Exp,
                                bias=cbias,
                                scale=EXP_SCALE,
                            )
                        for hj in range(HI):
                            nc.tensor.matmul(
                                ctx_ps[hj],
                                v_sb[:, 2 * pr : 2 * pr + 2, hj * P : (hj + 1) * P],
                                ex2[:],
                                start=(pr == 0),
                                stop=(pr == NKT // 2 - 1),
                                perf_mode=DR,
                            )
                        nc.tensor.matmul(
                            den,
                            ones8[:, :, 0:1],
                            ex2[:],
                            start=(pr == 0),
                            stop=(pr == NKT // 2 - 1),
                            perf_mode=DR,
                        )

                    # evacuate raw (unnormalized) ctx' as bf16 so Wo can start
                    # immediately; the 1/den scaling is applied after Wo (Wo is
                    # linear in ctx), off the PE critical path.
                    ctxb = []
                    for hj in range(HI):
                        cb = p2.tile([P, QB], bf16, name="cb", tag="ctxb", bufs=8)
                        if hj % 2 == 0:
                            nc.scalar.copy(cb, ctx_ps[hj])
                        else:
                            nc.vector.tensor_copy(cb, ctx_ps[hj])
                        ctxb.append(cb)

                    # prefetch next block's Q projection to keep PE busy
                    # through the normalize chain below
                    if qb + 1 < NQB:
                        pipe[qb + 1] = q_proj(qb + 1)
                    rec1 = p2.tile([1, QB], f32, name="rec1", tag="rec1", bufs=2)
                    nc.vector.reciprocal(rec1, den)
                    rps = psum.tile([P, QB], f32, name="rps", tag="a", bufs=3)
                    nc.tensor.matmul(rps, ones_f32[:], rec1)
                    rec128 = p2.tile([P, QB], f32, name="rec128", tag="rec128", bufs=2)
                    nc.scalar.copy(rec128, rps)

                    # Wo then scale by 1/(16 den), + bias + residual -> y^T [e, q]
                    ys = []
                    for ej in range(EI):
                        po = psum.tile([P, QB], f32, name="po", tag="a", bufs=3)
                        for hj in range(HI):
                            nc.tensor.matmul(
                                po,
                                wo_sb[:, hj : hj + 1, ej * P : (ej + 1) * P],
                                ctxb[hj],
                                start=(hj == 0),
                                stop=(hj == HI - 1),
                            )
                        yn = p2.tile([P, QB], f32, name="yn", tag="yn", bufs=3)
                        nc.vector.tensor_tensor(yn, po, rec128, op=ALU.mult)
                        y1 = p2.tile([P, QB], bf16, name="y1", tag="y1", bufs=3)
                        nc.scalar.add(y1, yn, bo_sb[:, ej : ej + 1])
                        y = p2.tile([P, QB], bf16, name="y", tag="y", bufs=16)
                        nc.vector.tensor_tensor(y, y1, qt[:, ej : ej + 1, :], op=ALU.add)
                        ys.append(y)

                    # Wfc back to natural [q, f], then LayerNorm + out
                    for qi in range(QB // P):
                        o2 = p2.tile([P, E], f32, name="o2", tag="o2", bufs=2)
                        for fj in range(2):
                            pf = psum.tile([P, H], f32, name="pf", tag="a", bufs=3)
                            for ej in range(EI):
                                nc.tensor.matmul(
                                    pf,
                                    ys[ej][:, qi * P : (qi + 1) * P],
                                    wfc_sb[:, ej : ej + 1, fj * H : (fj + 1) * H],
                                    start=(ej == 0),
                                    stop=(ej == EI - 1),
                                )
                            nc.vector.tensor_copy(o2[:, fj * H : (fj + 1) * H], pf)
                        st6 = p2.tile([P, 2, 6], f32, name="st6", tag="st6", bufs=3)
                        for g in range(2):
                            nc.vector.bn_stats(
                                st6[:, g : g + 1, :], o2[:, g * H : (g + 1) * H]
                            )
                        st2 = p2.tile([P, 2], f32, name="st2", tag="st2", bufs=3)
                        nc.vector.bn_aggr(st2, st6.rearrange("p a b -> p (a b)"))
                        stdt = p2.tile([P, 1], f32, name="stdt", tag="stdt", bufs=3)
                        nc.scalar.activation(stdt, st2[:, 1:2], AF.Sqrt, bias=eps_sb)
                        rstd = p2.tile([P, 1], f32, name="rstd", tag="rstd", bufs=3)
                        nc.vector.reciprocal(rstd, stdt)
                        nmr = p2.tile([P, 1], f32, name="nmr", tag="nmr", bufs=3)
                        nc.vector.tensor_tensor(nmr, st2[:, 0:1], rstd, op=ALU.mult)
                        nc.vector.tensor_scalar_mul(nmr, nmr, -1.0)
                        nrm = p2.tile([P, E], f32, name="nrm", tag="nrm", bufs=2)
                        nc.scalar.activation(nrm, o2, AF.Identity, bias=nmr, scale=rstd)
                        outt = p2.tile([P, E], f32, name="outt", tag="outt", bufs=2)
                        nc.vector.tensor_tensor(outt, nrm, g128, op=ALU.mult)
                        nc.vector.tensor_tensor(outt, outt, b128, op=ALU.add)
                        r0 = qb * QB + qi * P
                        nc.sync.dma_start(out_d[r0 : r0 + P, :], outt)

    nc.compile()
    _cached_nc[repeat] = nc
    return nc


def _in_maps(q_feat, kv_feat, Wq, Wk, Wv, Wo, bo, Wfc, ln_gamma, ln_beta):
    maps = []
    for c in range(8):
        b, half = c // 2, c % 2
        maps.append(
            {
                "q_loc": np.ascontiguousarray(
                    q_feat[b, half * SQ : (half + 1) * SQ], dtype=np.float32
                ),
                "kv_loc": np.ascontiguousarray(kv_feat[b], dtype=np.float32),
                "Wq": np.asarray(Wq, np.float32),
                "Wk": np.asarray(Wk, np.float32),
                "Wv": np.asarray(Wv, np.float32),
                "Wo": np.asarray(Wo, np.float32),
                "bo": np.asarray(bo, np.float32),
                "Wfc": np.asarray(Wfc, np.float32),
                "ln_gamma": np.asarray(ln_gamma, np.float32),
                "ln_beta": np.asarray(ln_beta, np.float32),
            }
        )
    return maps


def run_spmd(inputs, repeat=1, **kwargs):
    """Run the SPMD kernel; returns (full_output, BassKernelResults)."""
    nc = _build(repeat)
    maps = _in_maps(**inputs)
    res = bass_utils.run_bass_kernel_spmd(nc, maps, core_ids=list(range(8)), **kwargs)
    out = np.empty((4, 2 * SQ, E), np.float32)
    for c in range(8):
        b, half = c // 2, c % 2
        out[b, half * SQ : (half + 1) * SQ] = res.results[c]["out_loc"]
    return out, res


def kernel(**inputs):
    out, _ = run_spmd(inputs)
    return out
